# revision 25
# baseline (speedup 1.0000x reference)
"""Trainium2 Bass kernel for nn_Dropout_Conv2D (per-sample dropout-masked 3x3 conv).

Problem: y[b] = conv2d(x[b], weight * (u_w[b] > p)) + bias * (u_b[b] > p)
  x [32,128,64,64] f32, weight [256,128,3,3] f32, bias [256] f32,
  u_w [32,256,128,3,3] f32, u_b [32,256] f32  ->  y [32,256,64,64] f32

Strategy: data-parallel over batch, 4 samples per NeuronCore across 8 cores.
Per core, each sample's conv runs as 9 shift-offset matmuls accumulated in
PSUM: lhsT = masked weight slice [Cin=128, Cout-half=128] (fp16), rhs = a
[128, 8rows x 64cols] window of the zero-padded input image held in SBUF with
row stride 66. Dropout masks are computed on-device in fp32 (exact compare),
matmul inputs are fp16 (PE full rate), accumulation is fp32 in PSUM.

This file is self-contained: shapes/sharding are hardcoded to the problem.
"""
import numpy as np
from contextlib import ExitStack

import concourse.bass as bass
import concourse.tile as tile
from concourse import bacc, mybir
from concourse.alu_op_type import AluOpType
from concourse.tile_rust import add_dep_helper
from concourse.bass_utils import run_bass_kernel_spmd

P_DROP = 0.2
NCORES = 8
B = 32
S = B // NCORES   # samples per core
CIN = 128
COUT = 256
H = 64
W = 64
NK = 9            # 3x3 kernel positions
CHROWS = 8        # output rows per 512-pixel matmul chunk
ROWSTRIDE = 66    # padded image row stride in SBUF
WARMUP = 200      # PE warmup matmuls (HAM clock-gate ramp)

_compiled = None  # cache (nc) across calls


def _build():
    FP32 = mybir.dt.float32
    DT = mybir.dt.float16
    NCHUNK = H // CHROWS
    PADF = (H + 2) * ROWSTRIDE

    nc = bacc.Bacc("TRN2", target_bir_lowering=False, debug=False)
    xs = nc.dram_tensor("xs", (S, CIN, H * W), DT, kind="ExternalInput").ap()
    # weight/uniform layouts are m-major (free idx = m*1152 + k*128 + o'), so
    # the first cout-half group only depends on the first half of each tensor
    # and each half is a fully contiguous DRAM read
    wT = nc.dram_tensor("wT", (2, 128, NK * 128), DT, kind="ExternalInput").ap()
    # uw blocks are (m-half, k-triple)-major: each [128, 384] block contiguous
    uw = nc.dram_tensor("uw", (S, 2, 3, 128, 3 * 128), FP32, kind="ExternalInput").ap()
    ubb = nc.dram_tensor("ubb", (128, 4 * S), FP32, kind="ExternalInput").ap()
    y = nc.dram_tensor("y", (S, 2, 128, H * W), FP32, kind="ExternalOutput").ap()
    MH = NK * 128  # 1152 cols per m-half

    with tile.TileContext(nc) as tc:
        ctx = ExitStack()
        consts = ctx.enter_context(tc.tile_pool(name="consts", bufs=1))
        inbuf = ctx.enter_context(tc.tile_pool(name="inbuf", bufs=2))
        wmpool = ctx.enter_context(tc.tile_pool(name="wm", bufs=2))
        xpads = ctx.enter_context(tc.tile_pool(name="xpads", bufs=1))
        outp = ctx.enter_context(tc.tile_pool(name="outp", bufs=2))
        psum = ctx.enter_context(tc.tile_pool(name="psum", bufs=4, space=bass.MemorySpace.PSUM))

        # PE warmup from a zeroed tile: keeps the HAM clock gate open while
        # the input DMAs land, so the real matmul stream starts at 2.4 GHz.
        wu_src = consts.tile([128, 128], DT, tag="wu_src")
        nc.vector.memset(wu_src[:], 0.0)
        wu_ps = psum.tile([128, 1024], FP32, tag="ps", name="warm_ps")
        warm_mms = []
        for i in range(WARMUP):
            warm_mms.append(nc.tensor.matmul(wu_ps[:, :64], wu_src[:], wu_src[:, :64],
                                             start=(i == 0), stop=(i == WARMUP - 1)))

        # merged tiny bias input [u_b | bias] and the transposed weight, the
        # m=0 half first — it is all the first half-group needs
        ubb_sb = consts.tile([128, 4 * S], FP32, tag="ubb")
        nc.sync.dma_start(ubb_sb[:], ubb[:])
        wT_sb = consts.tile([128, 2 * MH], DT, tag="wT")
        wt_dma0 = nc.sync.dma_start(wT_sb[:, :MH], wT[0])
        bper = consts.tile([128, 2 * S], FP32, tag="bper")
        nc.vector.tensor_scalar(bper[:], ubb_sb[:, :2 * S], P_DROP, None, AluOpType.is_gt)
        nc.vector.tensor_tensor(bper[:], bper[:], ubb_sb[:, 2 * S:], AluOpType.mult)

        # two padded-image buffers; only the zero border is memset once —
        # sample interiors overwrite [1:H+1, 1:W+1] and the border persists
        xpad_tiles = []
        for i in range(2):
            t = xpads.tile([128, PADF], DT, tag=f"xpad{i}")
            t3 = t.rearrange("p (h w) -> p h w", h=H + 2, w=ROWSTRIDE)
            nc.gpsimd.memset(t3[:, 0, :W + 2], 0.0)
            nc.gpsimd.memset(t3[:, H + 1, :W + 2], 0.0)
            nc.gpsimd.memset(t3[:, :, 0], 0.0)
            nc.gpsimd.memset(t3[:, :, W + 1], 0.0)
            xpad_tiles.append(t3)

        # sample 0: x in two halves on the scalar ring (top rows feed the
        # first half-group); uw m=0 half right behind wT's m=0 half on sync
        XS = 33 * W
        xf0 = inbuf.tile([128, H * W], DT, tag="xf", name="xf0")
        nc.scalar.dma_start(xf0[:, :XS], xs[0][:, :XS])
        nc.scalar.dma_start(xf0[:, XS:], xs[0][:, XS:])
        uw0 = inbuf.tile([128, 2 * MH], FP32, tag="uw", name="uw0")
        wm0 = wmpool.tile([128, 2 * MH], DT, tag="wm", name="wm0")
        KB = 3 * 128  # 384-col k-triple block
        uw0m0_dma = nc.sync.dma_start(uw0[:, :KB], uw[0][0][0])
        nc.sync.dma_start(uw0[:, KB:2 * KB], uw[0][0][1])
        nc.sync.dma_start(uw0[:, 2 * KB:3 * KB], uw[0][0][2])
        nc.sync.dma_start(wT_sb[:, MH:], wT[1])
        for c in range(3):
            nc.sync.dma_start(uw0[:, MH + c * KB:MH + (c + 1) * KB], uw[0][1][c])
        # warmup batches paced by the critical loads: batch 2 bridges to the
        # m=0 weight landing, batch 3 starts when the m=0 uniforms land — the
        # same DMA the first mask op waits on — so the PE stays at 2.4GHz
        # right up to the real stream no matter how slow the DMAs are
        add_dep_helper(warm_mms[125].ins, wt_dma0.ins, sync=True,
                       reason="pace warmup to wT m0")
        add_dep_helper(warm_mms[155].ins, uw0m0_dma.ins, sync=True,
                       reason="pace warmup to uw m0")
        xp0 = xpad_tiles[0]
        xf0_3 = xf0.rearrange("p (h w) -> p h w", h=H, w=W)
        # m=0 masks pipelined per k-triple block; spreads interleaved
        for c in range(3):
            lo, hi = c * KB, (c + 1) * KB
            nc.vector.tensor_scalar(wm0[:, lo:hi], uw0[:, lo:hi], P_DROP, None, AluOpType.is_gt)
            nc.vector.tensor_tensor(wm0[:, lo:hi], wm0[:, lo:hi], wT_sb[:, lo:hi], AluOpType.mult)
            if c == 0:
                nc.vector.tensor_copy(xp0[:, 1:34, 1:W + 1], xf0_3[:, :33, :])
            elif c == 1:
                nc.vector.tensor_copy(xp0[:, 34:H + 1, 1:W + 1], xf0_3[:, 33:, :])
        nc.vector.tensor_scalar(wm0[:, MH:], uw0[:, MH:], P_DROP, None, AluOpType.is_gt)
        last_mask_op = nc.vector.tensor_tensor(wm0[:, MH:], wm0[:, MH:], wT_sb[:, MH:],
                                               AluOpType.mult)

        def load_next(b, gate):
            xf = inbuf.tile([128, H * W], DT, tag="xf", name=f"xf{b}")
            dx = nc.scalar.dma_start(xf[:], xs[b])
            uw_sb = inbuf.tile([128, 2 * MH], FP32, tag="uw", name=f"uw{b}")
            for mm in range(2):
                for c in range(3):
                    lo = mm * MH + c * KB
                    du = nc.sync.dma_start(uw_sb[:, lo:lo + KB], uw[b][mm][c])
                    if gate is not None:
                        add_dep_helper(du.ins, gate.ins, reason="delay prefetch")
            if gate is not None:
                # keep HBM bandwidth on sample 0's critical loads
                add_dep_helper(dx.ins, gate.ins, reason="delay prefetch")
            return xf, uw_sb

        cur = (wm0, xp0)
        nxt_raw = None
        for b in range(S):
            wm_sb, xp3 = cur
            if b + 1 < S:
                nxt_raw = load_next(b + 1, last_mask_op if b == 0 else None)

            for m in range(2):
                bias_ap = bper[:, b * 2 + m: b * 2 + m + 1]
                out_sb = outp.tile([128, H * W], FP32, tag="out", name=f"out{b}_{m}")
                # two half-groups of 4 chunks: PSUM banks drain mid-group, so
                # the next group's start=True matmuls never wait on a drain
                for half in range(2):
                    pts = [psum.tile([128, 1024], FP32, tag="ps",
                                     name=f"ps_{b}_{m}_{half}_{j}") for j in range(2)]
                    for k in range(NK):
                        ky, kx = divmod(k, 3)
                        lhsT = wm_sb[:, m * MH + k * 128: m * MH + (k + 1) * 128]
                        for i in range(4):
                            r0 = (half * 4 + i) * CHROWS
                            rhs = xp3[:, r0 + ky: r0 + ky + CHROWS, kx: kx + W]
                            nc.tensor.matmul(pts[i // 2][:, (i % 2) * 512:(i % 2) * 512 + 512],
                                             lhsT, rhs, start=(k == 0), stop=(k == NK - 1))
                    for j in range(2):
                        lo = (half * 2 + j) * 1024
                        nc.vector.tensor_scalar(out_sb[:, lo:lo + 512], pts[j][:, :512],
                                                bias_ap, None, AluOpType.add)
                        nc.scalar.activation(out_sb[:, lo + 512:lo + 1024], pts[j][:, 512:],
                                             mybir.ActivationFunctionType.Identity,
                                             bias=bias_ap, scale=1.0)
                        nc.scalar.dma_start(y[b, m, :, lo:lo + 1024], out_sb[:, lo:lo + 1024])

                if m == 0 and b + 1 < S:
                    xf_n, uw_n = nxt_raw
                    wm_n = wmpool.tile([128, NK * COUT], DT, tag="wm", name=f"wm{b+1}")
                    nc.vector.tensor_scalar(wm_n[:], uw_n[:], P_DROP, None, AluOpType.is_gt)
                    nc.vector.tensor_tensor(wm_n[:], wm_n[:], wT_sb[:], AluOpType.mult)
                    xp_n = xpad_tiles[(b + 1) % 2]
                    nc.vector.tensor_copy(
                        xp_n[:, 1:H + 1, 1:W + 1],
                        xf_n.rearrange("p (h w) -> p h w", h=H, w=W)[:],
                    )
                    cur = (wm_n, xp_n)
        ctx.close()
    nc.compile()
    return nc


def _prep_core_inputs(x, weight_f16T, u_w, ubb_h):
    """Host-side layout prep for one core's shard (no compute, layout only).

    m-major uniform layout: uw[s, m, i, k*128+o'] = u_w[s, m*128+o', i, ky, kx]."""
    xs_h = x.reshape(S, CIN, H * W).astype(np.float16)
    uw_h = np.ascontiguousarray(
        u_w.reshape(S, 2, 128, CIN, 3, 3).transpose(0, 1, 3, 4, 5, 2)
        .reshape(S, 2, CIN, 3, 3 * 128).transpose(0, 1, 3, 2, 4))
    return {"xs": xs_h, "wT": weight_f16T, "uw": uw_h, "ubb": ubb_h}


def _prep_in_maps(x, weight, bias, u_w, u_b):
    """Build the 8 per-core input maps (shared by kernel() and test harnesses)."""
    wT_h = np.ascontiguousarray(
        weight.reshape(2, 128, CIN, 3, 3).transpose(0, 2, 3, 4, 1)
        .reshape(2, CIN, NK * 128)).astype(np.float16)
    bb = np.ascontiguousarray(bias.reshape(2, 128).T)          # [128, 2]
    bbr_h = np.tile(bb, (1, S))                                # [128, 2S]
    in_maps = []
    for c in range(NCORES):
        sl = slice(c * S, (c + 1) * S)
        ubr_h = np.ascontiguousarray(
            u_b[sl].reshape(S, 2, 128).transpose(2, 0, 1).reshape(128, S * 2))
        ubb_h = np.concatenate([ubr_h, bbr_h], axis=1)         # [128, 4S]
        in_maps.append(_prep_core_inputs(x[sl], wT_h, u_w[sl], ubb_h))
    return in_maps


def kernel(x, weight, bias, u_w, u_b):
    global _compiled
    x = np.ascontiguousarray(np.asarray(x, dtype=np.float32))
    weight = np.asarray(weight, dtype=np.float32)
    bias = np.asarray(bias, dtype=np.float32)
    u_w = np.ascontiguousarray(np.asarray(u_w, dtype=np.float32))
    u_b = np.asarray(u_b, dtype=np.float32)

    if _compiled is None:
        _compiled = _build()
    nc = _compiled

    in_maps = _prep_in_maps(x, weight, bias, u_w, u_b)
    res = run_bass_kernel_spmd(nc, in_maps, core_ids=list(range(NCORES)))
    y = np.concatenate([r["y"].reshape(S, COUT, H, W) for r in res.results], axis=0)
    return y


# revision 26
# speedup vs baseline: 1.0019x; 1.0019x over previous
"""Trainium2 Bass kernel for nn_Dropout_Conv2D (per-sample dropout-masked 3x3 conv).

Problem: y[b] = conv2d(x[b], weight * (u_w[b] > p)) + bias * (u_b[b] > p)
  x [32,128,64,64] f32, weight [256,128,3,3] f32, bias [256] f32,
  u_w [32,256,128,3,3] f32, u_b [32,256] f32  ->  y [32,256,64,64] f32

Strategy: data-parallel over batch, 4 samples per NeuronCore across 8 cores.
Per core, each sample's conv runs as 9 shift-offset matmuls accumulated in
PSUM: lhsT = masked weight slice [Cin=128, Cout-half=128] (fp16), rhs = a
[128, 8rows x 64cols] window of the zero-padded input image held in SBUF with
row stride 66. Dropout masks are computed on-device in fp32 (exact compare),
matmul inputs are fp16 (PE full rate), accumulation is fp32 in PSUM.

This file is self-contained: shapes/sharding are hardcoded to the problem.
"""
import numpy as np
from contextlib import ExitStack

import concourse.bass as bass
import concourse.tile as tile
from concourse import bacc, mybir
from concourse.alu_op_type import AluOpType
from concourse.tile_rust import add_dep_helper
from concourse.bass_utils import run_bass_kernel_spmd

P_DROP = 0.2
NCORES = 8
B = 32
S = B // NCORES   # samples per core
CIN = 128
COUT = 256
H = 64
W = 64
NK = 9            # 3x3 kernel positions
CHROWS = 8        # output rows per 512-pixel matmul chunk
ROWSTRIDE = 66    # padded image row stride in SBUF
WARMUP = 200      # PE warmup matmuls (HAM clock-gate ramp)

_compiled = None  # cache (nc) across calls


def _build():
    FP32 = mybir.dt.float32
    DT = mybir.dt.float16
    NCHUNK = H // CHROWS
    PADF = (H + 2) * ROWSTRIDE

    nc = bacc.Bacc("TRN2", target_bir_lowering=False, debug=False)
    xs = nc.dram_tensor("xs", (S, CIN, H * W), DT, kind="ExternalInput").ap()
    # weight/uniform layouts are m-major (free idx = m*1152 + k*128 + o'), so
    # the first cout-half group only depends on the first half of each tensor
    # and each half is a fully contiguous DRAM read
    wT = nc.dram_tensor("wT", (2, 128, NK * 128), DT, kind="ExternalInput").ap()
    # uw blocks are (m-half, k-triple)-major: each [128, 384] block contiguous
    uw = nc.dram_tensor("uw", (S, 2, 3, 128, 3 * 128), FP32, kind="ExternalInput").ap()
    ubb = nc.dram_tensor("ubb", (128, 4 * S), FP32, kind="ExternalInput").ap()
    y = nc.dram_tensor("y", (S, 2, 128, H * W), FP32, kind="ExternalOutput").ap()
    MH = NK * 128  # 1152 cols per m-half

    with tile.TileContext(nc) as tc:
        ctx = ExitStack()
        consts = ctx.enter_context(tc.tile_pool(name="consts", bufs=1))
        inbuf = ctx.enter_context(tc.tile_pool(name="inbuf", bufs=2))
        wmpool = ctx.enter_context(tc.tile_pool(name="wm", bufs=2))
        xpads = ctx.enter_context(tc.tile_pool(name="xpads", bufs=1))
        outp = ctx.enter_context(tc.tile_pool(name="outp", bufs=2))
        psum = ctx.enter_context(tc.tile_pool(name="psum", bufs=4, space=bass.MemorySpace.PSUM))

        # PE warmup from a zeroed tile: keeps the HAM clock gate open while
        # the input DMAs land, so the real matmul stream starts at 2.4 GHz.
        wu_src = consts.tile([128, 128], DT, tag="wu_src")
        nc.vector.memset(wu_src[:], 0.0)
        wu_ps = psum.tile([128, 1024], FP32, tag="psA", bufs=2, name="warm_ps")
        warm_mms = []
        for i in range(WARMUP):
            warm_mms.append(nc.tensor.matmul(wu_ps[:, :64], wu_src[:], wu_src[:, :64],
                                             start=(i == 0), stop=(i == WARMUP - 1)))

        # merged tiny bias input [u_b | bias] and the transposed weight, the
        # m=0 half first — it is all the first half-group needs
        ubb_sb = consts.tile([128, 4 * S], FP32, tag="ubb")
        nc.sync.dma_start(ubb_sb[:], ubb[:])
        wT_sb = consts.tile([128, 2 * MH], DT, tag="wT")
        wt_dma0 = nc.sync.dma_start(wT_sb[:, :MH], wT[0])
        bper = consts.tile([128, 2 * S], FP32, tag="bper")
        nc.vector.tensor_scalar(bper[:], ubb_sb[:, :2 * S], P_DROP, None, AluOpType.is_gt)
        nc.vector.tensor_tensor(bper[:], bper[:], ubb_sb[:, 2 * S:], AluOpType.mult)

        # two padded-image buffers; only the zero border is memset once —
        # sample interiors overwrite [1:H+1, 1:W+1] and the border persists
        xpad_tiles = []
        for i in range(2):
            t = xpads.tile([128, PADF], DT, tag=f"xpad{i}")
            t3 = t.rearrange("p (h w) -> p h w", h=H + 2, w=ROWSTRIDE)
            nc.gpsimd.memset(t3[:, 0, :W + 2], 0.0)
            nc.gpsimd.memset(t3[:, H + 1, :W + 2], 0.0)
            nc.gpsimd.memset(t3[:, :, 0], 0.0)
            nc.gpsimd.memset(t3[:, :, W + 1], 0.0)
            xpad_tiles.append(t3)

        # sample 0: x in two halves on the scalar ring (top rows feed the
        # first half-group); uw m=0 half right behind wT's m=0 half on sync
        XS = 33 * W
        xf0 = inbuf.tile([128, H * W], DT, tag="xf", name="xf0")
        nc.scalar.dma_start(xf0[:, :XS], xs[0][:, :XS])
        nc.scalar.dma_start(xf0[:, XS:], xs[0][:, XS:])
        uw0 = inbuf.tile([128, 2 * MH], FP32, tag="uw", name="uw0")
        wm0 = wmpool.tile([128, 2 * MH], DT, tag="wm", name="wm0")
        KB = 3 * 128  # 384-col k-triple block
        uw0m0_dma = nc.sync.dma_start(uw0[:, :KB], uw[0][0][0])
        nc.sync.dma_start(uw0[:, KB:2 * KB], uw[0][0][1])
        nc.sync.dma_start(uw0[:, 2 * KB:3 * KB], uw[0][0][2])
        nc.sync.dma_start(wT_sb[:, MH:], wT[1])
        for c in range(3):
            nc.sync.dma_start(uw0[:, MH + c * KB:MH + (c + 1) * KB], uw[0][1][c])
        # warmup batches paced by the critical loads: batch 2 bridges to the
        # m=0 weight landing, batch 3 starts when the m=0 uniforms land — the
        # same DMA the first mask op waits on — so the PE stays at 2.4GHz
        # right up to the real stream no matter how slow the DMAs are
        add_dep_helper(warm_mms[125].ins, wt_dma0.ins, sync=True,
                       reason="pace warmup to wT m0")
        add_dep_helper(warm_mms[155].ins, uw0m0_dma.ins, sync=True,
                       reason="pace warmup to uw m0")
        xp0 = xpad_tiles[0]
        xf0_3 = xf0.rearrange("p (h w) -> p h w", h=H, w=W)
        # m=0 masks pipelined per k-triple block; spreads interleaved
        for c in range(3):
            lo, hi = c * KB, (c + 1) * KB
            nc.vector.tensor_scalar(wm0[:, lo:hi], uw0[:, lo:hi], P_DROP, None, AluOpType.is_gt)
            nc.vector.tensor_tensor(wm0[:, lo:hi], wm0[:, lo:hi], wT_sb[:, lo:hi], AluOpType.mult)
            if c == 0:
                nc.vector.tensor_copy(xp0[:, 1:34, 1:W + 1], xf0_3[:, :33, :])
            elif c == 1:
                nc.vector.tensor_copy(xp0[:, 34:H + 1, 1:W + 1], xf0_3[:, 33:, :])
        nc.vector.tensor_scalar(wm0[:, MH:], uw0[:, MH:], P_DROP, None, AluOpType.is_gt)
        last_mask_op = nc.vector.tensor_tensor(wm0[:, MH:], wm0[:, MH:], wT_sb[:, MH:],
                                               AluOpType.mult)

        def load_next(b, gate):
            xf = inbuf.tile([128, H * W], DT, tag="xf", name=f"xf{b}")
            dx = nc.scalar.dma_start(xf[:], xs[b])
            uw_sb = inbuf.tile([128, 2 * MH], FP32, tag="uw", name=f"uw{b}")
            for mm in range(2):
                for c in range(3):
                    lo = mm * MH + c * KB
                    du = nc.sync.dma_start(uw_sb[:, lo:lo + KB], uw[b][mm][c])
                    if gate is not None:
                        add_dep_helper(du.ins, gate.ins, reason="delay prefetch")
            if gate is not None:
                # keep HBM bandwidth on sample 0's critical loads
                add_dep_helper(dx.ins, gate.ins, reason="delay prefetch")
            return xf, uw_sb

        cur = (wm0, xp0)
        nxt_raw = None
        for b in range(S):
            wm_sb, xp3 = cur
            if b + 1 < S:
                nxt_raw = load_next(b + 1, last_mask_op if b == 0 else None)

            for m in range(2):
                bias_ap = bper[:, b * 2 + m: b * 2 + m + 1]
                out_sb = outp.tile([128, H * W], FP32, tag="out", name=f"out{b}_{m}")
                # two half-groups of 4 chunks: PSUM banks drain mid-group, so
                # the next group's start=True matmuls never wait on a drain
                for half in range(2):
                    # separate slot tags per half: a new group's half-A tiles
                    # reuse banks drained mid-previous-group, never the banks
                    # whose drains are still in flight at the group boundary
                    tag = "psA" if half == 0 else "psB"
                    pts = [psum.tile([128, 1024], FP32, tag=tag, bufs=2,
                                     name=f"ps_{b}_{m}_{half}_{j}") for j in range(2)]
                    for k in range(NK):
                        ky, kx = divmod(k, 3)
                        lhsT = wm_sb[:, m * MH + k * 128: m * MH + (k + 1) * 128]
                        for i in range(4):
                            r0 = (half * 4 + i) * CHROWS
                            rhs = xp3[:, r0 + ky: r0 + ky + CHROWS, kx: kx + W]
                            nc.tensor.matmul(pts[i // 2][:, (i % 2) * 512:(i % 2) * 512 + 512],
                                             lhsT, rhs, start=(k == 0), stop=(k == NK - 1))
                    for j in range(2):
                        lo = (half * 2 + j) * 1024
                        nc.vector.tensor_scalar(out_sb[:, lo:lo + 512], pts[j][:, :512],
                                                bias_ap, None, AluOpType.add)
                        nc.scalar.activation(out_sb[:, lo + 512:lo + 1024], pts[j][:, 512:],
                                             mybir.ActivationFunctionType.Identity,
                                             bias=bias_ap, scale=1.0)
                        nc.scalar.dma_start(y[b, m, :, lo:lo + 1024], out_sb[:, lo:lo + 1024])

                if m == 0 and b + 1 < S:
                    xf_n, uw_n = nxt_raw
                    wm_n = wmpool.tile([128, NK * COUT], DT, tag="wm", name=f"wm{b+1}")
                    nc.vector.tensor_scalar(wm_n[:], uw_n[:], P_DROP, None, AluOpType.is_gt)
                    nc.vector.tensor_tensor(wm_n[:], wm_n[:], wT_sb[:], AluOpType.mult)
                    xp_n = xpad_tiles[(b + 1) % 2]
                    nc.vector.tensor_copy(
                        xp_n[:, 1:H + 1, 1:W + 1],
                        xf_n.rearrange("p (h w) -> p h w", h=H, w=W)[:],
                    )
                    cur = (wm_n, xp_n)
        ctx.close()
    nc.compile()
    return nc


def _prep_core_inputs(x, weight_f16T, u_w, ubb_h):
    """Host-side layout prep for one core's shard (no compute, layout only).

    m-major uniform layout: uw[s, m, i, k*128+o'] = u_w[s, m*128+o', i, ky, kx]."""
    xs_h = x.reshape(S, CIN, H * W).astype(np.float16)
    uw_h = np.ascontiguousarray(
        u_w.reshape(S, 2, 128, CIN, 3, 3).transpose(0, 1, 3, 4, 5, 2)
        .reshape(S, 2, CIN, 3, 3 * 128).transpose(0, 1, 3, 2, 4))
    return {"xs": xs_h, "wT": weight_f16T, "uw": uw_h, "ubb": ubb_h}


def _prep_in_maps(x, weight, bias, u_w, u_b):
    """Build the 8 per-core input maps (shared by kernel() and test harnesses)."""
    wT_h = np.ascontiguousarray(
        weight.reshape(2, 128, CIN, 3, 3).transpose(0, 2, 3, 4, 1)
        .reshape(2, CIN, NK * 128)).astype(np.float16)
    bb = np.ascontiguousarray(bias.reshape(2, 128).T)          # [128, 2]
    bbr_h = np.tile(bb, (1, S))                                # [128, 2S]
    in_maps = []
    for c in range(NCORES):
        sl = slice(c * S, (c + 1) * S)
        ubr_h = np.ascontiguousarray(
            u_b[sl].reshape(S, 2, 128).transpose(2, 0, 1).reshape(128, S * 2))
        ubb_h = np.concatenate([ubr_h, bbr_h], axis=1)         # [128, 4S]
        in_maps.append(_prep_core_inputs(x[sl], wT_h, u_w[sl], ubb_h))
    return in_maps


def kernel(x, weight, bias, u_w, u_b):
    global _compiled
    x = np.ascontiguousarray(np.asarray(x, dtype=np.float32))
    weight = np.asarray(weight, dtype=np.float32)
    bias = np.asarray(bias, dtype=np.float32)
    u_w = np.ascontiguousarray(np.asarray(u_w, dtype=np.float32))
    u_b = np.asarray(u_b, dtype=np.float32)

    if _compiled is None:
        _compiled = _build()
    nc = _compiled

    in_maps = _prep_in_maps(x, weight, bias, u_w, u_b)
    res = run_bass_kernel_spmd(nc, in_maps, core_ids=list(range(NCORES)))
    y = np.concatenate([r["y"].reshape(S, COUT, H, W) for r in res.results], axis=0)
    return y


# revision 27
# speedup vs baseline: 1.0039x; 1.0020x over previous
"""Trainium2 Bass kernel for nn_Dropout_Conv2D (per-sample dropout-masked 3x3 conv).

Problem: y[b] = conv2d(x[b], weight * (u_w[b] > p)) + bias * (u_b[b] > p)
  x [32,128,64,64] f32, weight [256,128,3,3] f32, bias [256] f32,
  u_w [32,256,128,3,3] f32, u_b [32,256] f32  ->  y [32,256,64,64] f32

Strategy: data-parallel over batch, 4 samples per NeuronCore across 8 cores.
Per core, each sample's conv runs as 9 shift-offset matmuls accumulated in
PSUM: lhsT = masked weight slice [Cin=128, Cout-half=128] (fp16), rhs = a
[128, 8rows x 64cols] window of the zero-padded input image held in SBUF with
row stride 66. Dropout masks are computed on-device in fp32 (exact compare),
matmul inputs are fp16 (PE full rate), accumulation is fp32 in PSUM.

This file is self-contained: shapes/sharding are hardcoded to the problem.
"""
import numpy as np
from contextlib import ExitStack

import concourse.bass as bass
import concourse.tile as tile
from concourse import bacc, mybir
from concourse.alu_op_type import AluOpType
from concourse.tile_rust import add_dep_helper
from concourse.bass_utils import run_bass_kernel_spmd

P_DROP = 0.2
NCORES = 8
B = 32
S = B // NCORES   # samples per core
CIN = 128
COUT = 256
H = 64
W = 64
NK = 9            # 3x3 kernel positions
CHROWS = 8        # output rows per 512-pixel matmul chunk
ROWSTRIDE = 66    # padded image row stride in SBUF
WARMUP = 200      # PE warmup matmuls (HAM clock-gate ramp)

_compiled = None  # cache (nc) across calls


def _build():
    FP32 = mybir.dt.float32
    DT = mybir.dt.float16
    NCHUNK = H // CHROWS
    PADF = (H + 2) * ROWSTRIDE

    nc = bacc.Bacc("TRN2", target_bir_lowering=False, debug=False)
    xs = nc.dram_tensor("xs", (S, CIN, H * W), DT, kind="ExternalInput").ap()
    # weight/uniform layouts are m-major (free idx = m*1152 + k*128 + o'), so
    # the first cout-half group only depends on the first half of each tensor
    # and each half is a fully contiguous DRAM read
    wT = nc.dram_tensor("wT", (2, 128, NK * 128), DT, kind="ExternalInput").ap()
    # uw blocks are (m-half, k-triple)-major: each [128, 384] block contiguous
    uw = nc.dram_tensor("uw", (S, 2, 3, 128, 3 * 128), FP32, kind="ExternalInput").ap()
    ubb = nc.dram_tensor("ubb", (128, 4 * S), FP32, kind="ExternalInput").ap()
    y = nc.dram_tensor("y", (S, 2, 128, H * W), FP32, kind="ExternalOutput").ap()
    MH = NK * 128  # 1152 cols per m-half

    with tile.TileContext(nc) as tc:
        ctx = ExitStack()
        consts = ctx.enter_context(tc.tile_pool(name="consts", bufs=1))
        inbuf = ctx.enter_context(tc.tile_pool(name="inbuf", bufs=2))
        wmpool = ctx.enter_context(tc.tile_pool(name="wm", bufs=2))
        xpads = ctx.enter_context(tc.tile_pool(name="xpads", bufs=1))
        outp = ctx.enter_context(tc.tile_pool(name="outp", bufs=2))
        psum = ctx.enter_context(tc.tile_pool(name="psum", bufs=4, space=bass.MemorySpace.PSUM))

        # PE warmup from a zeroed tile: keeps the HAM clock gate open while
        # the input DMAs land, so the real matmul stream starts at 2.4 GHz.
        wu_src = consts.tile([128, 128], DT, tag="wu_src")
        nc.vector.memset(wu_src[:], 0.0)
        wu_ps = psum.tile([128, 1024], FP32, tag="psA", bufs=2, name="warm_ps")
        warm_mms = []
        for i in range(WARMUP):
            warm_mms.append(nc.tensor.matmul(wu_ps[:, :64], wu_src[:], wu_src[:, :64],
                                             start=(i == 0), stop=(i == WARMUP - 1)))

        # merged tiny bias input [u_b | bias] and the transposed weight, the
        # m=0 half first — it is all the first half-group needs
        ubb_sb = consts.tile([128, 4 * S], FP32, tag="ubb")
        nc.sync.dma_start(ubb_sb[:], ubb[:])
        wT_sb = consts.tile([128, 2 * MH], DT, tag="wT")
        wt_dma0 = nc.sync.dma_start(wT_sb[:, :MH], wT[0])
        bper = consts.tile([128, 2 * S], FP32, tag="bper")
        nc.vector.tensor_scalar(bper[:], ubb_sb[:, :2 * S], P_DROP, None, AluOpType.is_gt)
        nc.vector.tensor_tensor(bper[:], bper[:], ubb_sb[:, 2 * S:], AluOpType.mult)

        # two padded-image buffers; only the zero border is memset once —
        # sample interiors overwrite [1:H+1, 1:W+1] and the border persists
        xpad_tiles = []
        for i in range(2):
            t = xpads.tile([128, PADF], DT, tag=f"xpad{i}")
            t3 = t.rearrange("p (h w) -> p h w", h=H + 2, w=ROWSTRIDE)
            nc.gpsimd.memset(t3[:, 0, :W + 2], 0.0)
            nc.gpsimd.memset(t3[:, H + 1, :W + 2], 0.0)
            nc.gpsimd.memset(t3[:, :, 0], 0.0)
            nc.gpsimd.memset(t3[:, :, W + 1], 0.0)
            xpad_tiles.append(t3)

        # sample 0: x in two halves on the scalar ring (top rows feed the
        # first half-group); uw m=0 half right behind wT's m=0 half on sync
        XS = 33 * W
        xf0 = inbuf.tile([128, H * W], DT, tag="xf", name="xf0")
        nc.scalar.dma_start(xf0[:, :XS], xs[0][:, :XS])
        nc.scalar.dma_start(xf0[:, XS:], xs[0][:, XS:])
        uw0 = inbuf.tile([128, 2 * MH], FP32, tag="uw", name="uw0")
        wm0 = wmpool.tile([128, 2 * MH], DT, tag="wm", name="wm0")
        KB = 3 * 128  # 384-col k-triple block
        uw0m0_dma = nc.sync.dma_start(uw0[:, :KB], uw[0][0][0])
        nc.sync.dma_start(uw0[:, KB:2 * KB], uw[0][0][1])
        nc.sync.dma_start(uw0[:, 2 * KB:3 * KB], uw[0][0][2])
        nc.sync.dma_start(wT_sb[:, MH:], wT[1])
        for c in range(3):
            nc.sync.dma_start(uw0[:, MH + c * KB:MH + (c + 1) * KB], uw[0][1][c])
        # warmup batches paced by the critical loads: batch 2 bridges to the
        # m=0 weight landing, batch 3 starts when the m=0 uniforms land — the
        # same DMA the first mask op waits on — so the PE stays at 2.4GHz
        # right up to the real stream no matter how slow the DMAs are
        add_dep_helper(warm_mms[125].ins, wt_dma0.ins, sync=True,
                       reason="pace warmup to wT m0")
        add_dep_helper(warm_mms[155].ins, uw0m0_dma.ins, sync=True,
                       reason="pace warmup to uw m0")
        xp0 = xpad_tiles[0]
        xf0_3 = xf0.rearrange("p (h w) -> p h w", h=H, w=W)
        # m=0 masks pipelined per k-triple block; spreads interleaved
        for c in range(3):
            lo, hi = c * KB, (c + 1) * KB
            nc.vector.tensor_scalar(wm0[:, lo:hi], uw0[:, lo:hi], P_DROP, None, AluOpType.is_gt)
            nc.vector.tensor_tensor(wm0[:, lo:hi], wm0[:, lo:hi], wT_sb[:, lo:hi], AluOpType.mult)
            if c == 0:
                nc.vector.tensor_copy(xp0[:, 1:34, 1:W + 1], xf0_3[:, :33, :])
            elif c == 1:
                nc.vector.tensor_copy(xp0[:, 34:H + 1, 1:W + 1], xf0_3[:, 33:, :])
        nc.vector.tensor_scalar(wm0[:, MH:], uw0[:, MH:], P_DROP, None, AluOpType.is_gt)
        last_mask_op = nc.vector.tensor_tensor(wm0[:, MH:], wm0[:, MH:], wT_sb[:, MH:],
                                               AluOpType.mult)

        def load_next(b, gate):
            xf = inbuf.tile([128, H * W], DT, tag="xf", name=f"xf{b}")
            dx = nc.scalar.dma_start(xf[:], xs[b])
            uw_sb = inbuf.tile([128, 2 * MH], FP32, tag="uw", name=f"uw{b}")
            for mm in range(2):
                for c in range(3):
                    lo = mm * MH + c * KB
                    du = nc.sync.dma_start(uw_sb[:, lo:lo + KB], uw[b][mm][c])
                    if gate is not None:
                        add_dep_helper(du.ins, gate.ins, reason="delay prefetch")
            if gate is not None:
                # keep HBM bandwidth on sample 0's critical loads
                add_dep_helper(dx.ins, gate.ins, reason="delay prefetch")
            return xf, uw_sb

        cur = (wm0, xp0)
        nxt_raw = None
        for b in range(S):
            wm_sb, xp3 = cur
            if b + 1 < S:
                nxt_raw = load_next(b + 1, last_mask_op if b == 0 else None)

            for m in range(2):
                bias_ap = bper[:, b * 2 + m: b * 2 + m + 1]
                out_sb = outp.tile([128, H * W], FP32, tag="out", name=f"out{b}_{m}")
                # two half-groups of 4 chunks: PSUM banks drain mid-group, so
                # the next group's start=True matmuls never wait on a drain
                for half in range(2):
                    # separate slot tags per half: a new group's half-A tiles
                    # reuse banks drained mid-previous-group, never the banks
                    # whose drains are still in flight at the group boundary
                    tag = "psA" if half == 0 else "psB"
                    pts = [psum.tile([128, 1024], FP32, tag=tag, bufs=2,
                                     name=f"ps_{b}_{m}_{half}_{j}") for j in range(2)]
                    for k in range(NK):
                        ky, kx = divmod(k, 3)
                        lhsT = wm_sb[:, m * MH + k * 128: m * MH + (k + 1) * 128]
                        for i in range(4):
                            r0 = (half * 4 + i) * CHROWS
                            rhs = xp3[:, r0 + ky: r0 + ky + CHROWS, kx: kx + W]
                            nc.tensor.matmul(pts[i // 2][:, (i % 2) * 512:(i % 2) * 512 + 512],
                                             lhsT, rhs, start=(k == 0), stop=(k == NK - 1))
                    # both drains issue before any y-DMA: a DMA issue on the
                    # ACT sequencer (~0.6us) must not delay the second drain
                    for j in range(2):
                        lo = (half * 2 + j) * 1024
                        nc.vector.tensor_scalar(out_sb[:, lo:lo + 512], pts[j][:, :512],
                                                bias_ap, None, AluOpType.add)
                        nc.scalar.activation(out_sb[:, lo + 512:lo + 1024], pts[j][:, 512:],
                                             mybir.ActivationFunctionType.Identity,
                                             bias=bias_ap, scale=1.0)
                    for j in range(2):
                        lo = (half * 2 + j) * 1024
                        nc.scalar.dma_start(y[b, m, :, lo:lo + 1024], out_sb[:, lo:lo + 1024])

                if m == 0 and b + 1 < S:
                    xf_n, uw_n = nxt_raw
                    wm_n = wmpool.tile([128, NK * COUT], DT, tag="wm", name=f"wm{b+1}")
                    nc.vector.tensor_scalar(wm_n[:], uw_n[:], P_DROP, None, AluOpType.is_gt)
                    nc.vector.tensor_tensor(wm_n[:], wm_n[:], wT_sb[:], AluOpType.mult)
                    xp_n = xpad_tiles[(b + 1) % 2]
                    nc.vector.tensor_copy(
                        xp_n[:, 1:H + 1, 1:W + 1],
                        xf_n.rearrange("p (h w) -> p h w", h=H, w=W)[:],
                    )
                    cur = (wm_n, xp_n)
        ctx.close()
    nc.compile()
    return nc


def _prep_core_inputs(x, weight_f16T, u_w, ubb_h):
    """Host-side layout prep for one core's shard (no compute, layout only).

    m-major uniform layout: uw[s, m, i, k*128+o'] = u_w[s, m*128+o', i, ky, kx]."""
    xs_h = x.reshape(S, CIN, H * W).astype(np.float16)
    uw_h = np.ascontiguousarray(
        u_w.reshape(S, 2, 128, CIN, 3, 3).transpose(0, 1, 3, 4, 5, 2)
        .reshape(S, 2, CIN, 3, 3 * 128).transpose(0, 1, 3, 2, 4))
    return {"xs": xs_h, "wT": weight_f16T, "uw": uw_h, "ubb": ubb_h}


def _prep_in_maps(x, weight, bias, u_w, u_b):
    """Build the 8 per-core input maps (shared by kernel() and test harnesses)."""
    wT_h = np.ascontiguousarray(
        weight.reshape(2, 128, CIN, 3, 3).transpose(0, 2, 3, 4, 1)
        .reshape(2, CIN, NK * 128)).astype(np.float16)
    bb = np.ascontiguousarray(bias.reshape(2, 128).T)          # [128, 2]
    bbr_h = np.tile(bb, (1, S))                                # [128, 2S]
    in_maps = []
    for c in range(NCORES):
        sl = slice(c * S, (c + 1) * S)
        ubr_h = np.ascontiguousarray(
            u_b[sl].reshape(S, 2, 128).transpose(2, 0, 1).reshape(128, S * 2))
        ubb_h = np.concatenate([ubr_h, bbr_h], axis=1)         # [128, 4S]
        in_maps.append(_prep_core_inputs(x[sl], wT_h, u_w[sl], ubb_h))
    return in_maps


def kernel(x, weight, bias, u_w, u_b):
    global _compiled
    x = np.ascontiguousarray(np.asarray(x, dtype=np.float32))
    weight = np.asarray(weight, dtype=np.float32)
    bias = np.asarray(bias, dtype=np.float32)
    u_w = np.ascontiguousarray(np.asarray(u_w, dtype=np.float32))
    u_b = np.asarray(u_b, dtype=np.float32)

    if _compiled is None:
        _compiled = _build()
    nc = _compiled

    in_maps = _prep_in_maps(x, weight, bias, u_w, u_b)
    res = run_bass_kernel_spmd(nc, in_maps, core_ids=list(range(NCORES)))
    y = np.concatenate([r["y"].reshape(S, COUT, H, W) for r in res.results], axis=0)
    return y
